# revision 42
# baseline (speedup 1.0000x reference)
"""Trainium2 Bass kernel for nn_JointNet (RNN-T joint network).

Reference computation (fp32):
    enc_proj = encoder_outputs @ W1[:D]          # [B,T,H]
    dec_proj = decoder_outputs @ W1[D:]          # [B,U,H]
    hidden   = tanh(enc_proj[:,:,None,:] + dec_proj[:,None,:,:] + b1)
    out      = hidden @ W2                       # [B,T,U,V]

Shapes (hardcoded): B=4, T=256, U=64, D=512, H=512, V=1024.

Sharding: data-parallel over (B x T/2) -> 8 shards, one per NeuronCore.
Core c handles batch b = c//2, t-range [(c%2)*128, (c%2)*128+128).
No collectives needed; host assembles the output slices.

Numerics (max rel err ~1.7e-2 vs the 2e-2 gate, measured on the actual
seeded inputs; the computation is deterministic):
  - bf16 operands everywhere, fp32 PSUM accumulation, bf16 output
    (host upconverts).  bf16 matmul = 1 cycle/row on the PE, same as
    fp32r but without the free-dim>=256 restriction, and halves all
    DMA traffic.
  - The output GEMM contracts over H=512 in 4 K=128 chunks.  Two of
    the four chunks run as fp8 (e4m3) DoubleRow matmuls at 0.5
    cycles/row, using BOTH DoubleRow planes for error compensation:
      plane0: fp8(tanh/SW)      @ fp8(W2*SW)
      plane1: fp8(rho*SR)       @ fp8(W2*SW/SR),  rho = tanh/SW - plane0
    so the hid-side fp8 quantization error cancels to second order and
    only the W2-side fp8 error remains.
  - The host PERMUTES the H axis (W1 columns, b1, W2 rows - the output
    is invariant) so the 256 lanes with the smallest
    E[hidden^2]*||W2_row_fp8_err||^2 go to the fp8 chunks.

Per-core plan:
  1. PE warm-up: TRN2's PE clock ramps 0.65->1.2->2.4GHz, reaching full
     speed only after 3us of continuous execution; dummy matmuls keep
     the PE busy from ~0.5us so all real work runs at 2.4GHz.
  2. Load enc/dec PRE-TRANSPOSED into [d, t|u] layout via strided DMA
     access patterns (no PE transposes); W1/W2/b1 feature-on-partition.
     Spread over the SP/ACT/Pool queues in need-time order.
  3. Projections (bf16, fp32 psum): all-dec first (gates the tanh bias
     chain), then enc d-outer (consumes W1_enc chunks as they land).
  4. For each u: 4 tanh (ACT, bias trick), 2 Pool chains build the fp8
     planes, then per 512-wide v-chunk: 2 bf16 + 2 fp8-DoubleRow
     matmuls into one [128,1024] 2-bank psum tile; ONE DVE copy
     evacuates it (bf16) and one 256KB DMA per u streams out.
     Steady state: PE 1280ns/u, DVE 1192, ACT 1168, Pool ~600, SP 790.
  5. Tail: last u splits into 4 N=256 chunks, copies/DMAs fanned out so
     only a minimal final DMA is exposed.
"""

import numpy as np
import ml_dtypes

import concourse.bass as bass
import concourse.mybir as mybir
import concourse.tile as tile
from concourse.bass import ts
from concourse.bass_utils import run_bass_kernel_spmd
from concourse.vector_clock import ScopedClock

B, T, U, D, H, V = 4, 256, 64, 512, 512, 1024
T_SH = 128  # t-rows per core
N_CORES = 8
F32 = mybir.dt.float32
F32R = mybir.dt.float32r
BF = mybir.dt.bfloat16
F8 = mybir.dt.float8e4
P = 128
HT = H // P  # 4 h-tiles
DT = D // P  # 4 d-tiles
NF8 = 2      # h-chunks computed in fp8 DoubleRow
SW = 8.0     # W2 fp8 plane-0 scale
SR = 8.0     # residual plane scale (SR == SW -> 1-op residual on Pool)

_bf16 = ml_dtypes.bfloat16
_f8 = ml_dtypes.float8_e4m3


class _SingleWaitTileContext(tile.TileContext):
    """This container's walrus build accepts only ONE sync-wait per
    instruction ("Too many sync wait commands" at codegen otherwise).
    Peel extra waits onto same-engine no-ops emitted just before the
    real instruction, and chunk the kernel-tail drain the same way."""

    def _add_instruction(self, inst):
        si = inst.sync_info
        if si is not None and si.on_wait is not None and len(si.on_wait) > 1:
            waits = list(si.on_wait)
            for w in waits[:-1]:
                nop = mybir.InstNoOp(
                    name=self.nc.get_next_instruction_name(),
                    sync_info=mybir.SyncInfo(on_wait=[w], on_update=[]),
                    bass_nofuse=True,
                    engine=inst.engine,
                )
                super()._add_instruction(nop)
            inst.sync_info = mybir.SyncInfo(
                on_wait=[waits[-1]], on_update=list(si.on_update)
            )
        super()._add_instruction(inst)

    def _drain_and_barrier(self, tick_clock, wait_clock):
        nop0 = self.nc.sync.nop(nofuse=True)
        wait_clock.add_sem_waits(
            nop0.ins, ScopedClock({None: tick_clock.global_clock})
        )
        waits = list(nop0.ins.sync_info.on_wait)
        ups = list(nop0.ins.sync_info.on_update)
        nop0.ins.sync_info = mybir.SyncInfo(on_wait=waits[:1], on_update=ups)
        for w in waits[1:]:
            nxt = self.nc.sync.nop(nofuse=True)
            nxt.ins.sync_info = mybir.SyncInfo(on_wait=[w], on_update=[])
        self.nc.sync.drain()
        self.nc.all_engine_barrier()
        assert self.sems is not None
        popped = self.nc._tile_sem_poison_stack.pop()
        assert popped is self._sem_poison
        self.nc.clear_and_free_semaphores(list(self.sems.allocated().values()))
        self.nc.all_engine_barrier()


def build_nc():
    nc = bass.Bass(trn_type="TRN2")
    enc = nc.dram_tensor("enc", [T_SH, D], BF, kind="ExternalInput")
    dec = nc.dram_tensor("dec", [U, D], BF, kind="ExternalInput")
    w1 = nc.dram_tensor("w1", [2 * D, H], BF, kind="ExternalInput")
    b1 = nc.dram_tensor("b1", [H], F32, kind="ExternalInput")
    # fp8 W2 rows, chunks 0,1 (hid-compensated): [plane, rows, v]
    w28 = nc.dram_tensor("w28", [2, NF8 * P, V], F8, kind="ExternalInput")
    # fp8 W2 rows, chunks 2,3 (fully compensated): [0]=fp8(SW*W2),
    # [1]=fp8(SW*W2 - fp8(SW*W2)) (the weight residual)
    w28b = nc.dram_tensor("w28b", [2, 2 * P, V], F8, kind="ExternalInput")
    # u-major output: out[u] is one contiguous [T_SH, V] 256KB bf16 block.
    out = nc.dram_tensor("out", [U, T_SH, V], BF, kind="ExternalOutput")

    with _SingleWaitTileContext(nc) as tc:
        with (
            tc.tile_pool(name="consts", bufs=1) as consts,
            tc.tile_pool(name="hid", bufs=16) as hidp,
            tc.tile_pool(name="h8", bufs=8) as h8p,
            tc.tile_pool(name="ostage", bufs=4) as ostage,
            tc.tile_pool(name="prs", bufs=1, space="PSUM") as prs,
            tc.tile_pool(name="pso", bufs=3, space="PSUM") as pso,
        ):
            # Projection psum staging: 2 banks used alternately.  A
            # start=True in a bank marks the WHOLE 2KB zero-region pending,
            # so a bank can only be restarted after the previous result was
            # copied out - alternating two banks hides the copy latency.
            # 8 banks = 2 + pso 6.
            prA = prs.tile([P, T_SH], F32, tag="prA")
            prB = prs.tile([P, T_SH], F32, tag="prB")
            # ---- PE warm-up + ACT table preload ----
            # Dummies accumulate into the (not-yet-used) projection bank;
            # real projections later overwrite it with start=True.
            warm = consts.tile([P, 64], F32)
            nc.vector.memset(warm[:], 0.0)
            for _ in range(29):
                nc.tensor.matmul(
                    prA[:64, :64], warm[:].bitcast(F32R), warm[:].bitcast(F32R),
                    start=True, stop=True,
                )
            scrap = consts.tile([P, 1], F32)
            nc.gpsimd.memset(scrap[:], 0.0)
            nc.scalar.activation(
                scrap[:], scrap[:], mybir.ActivationFunctionType.Tanh
            )

            # ---- loads (need-time ordered across the 3 DMA queues) ----
            encT = consts.tile([P, T_SH, DT], BF)
            decT = consts.tile([P, U, DT], BF)
            w1_sb = consts.tile([P, 2 * DT, H], BF)  # [d_in, d_blk, h]
            w28d = consts.tile([P, NF8, 2, V], F8)  # [h_in, chunk, plane, v]
            w8d23 = consts.tile([P, 2, 2, V], F8)  # chunks 2,3 dup planes
            wr8d = consts.tile([P, 2, V], F8)  # (wr8_2, wr8_3) cross planes
            b1_sb = consts.tile([P, HT], F32)
            encr = enc.rearrange("t (o p) -> p t o", p=P)
            decr = dec.rearrange("u (o p) -> p u o", p=P)
            w1r = w1.rearrange("(o p) h -> p o h", p=P)
            w28r = w28.rearrange("pl (o p) v -> p pl o v", p=P)
            w28br = w28b.rearrange("pl (o p) v -> p pl o v", p=P)

            nc.sync.dma_start(decT[:], decr[:])
            nc.scalar.dma_start(w1_sb[:, DT : DT + 2], w1r[:, DT : DT + 2])
            nc.gpsimd.dma_start(w1_sb[:, DT + 2 :], w1r[:, DT + 2 :])
            nc.sync.dma_start(encT[:], encr[:])
            nc.scalar.dma_start(b1_sb[:], b1.rearrange("(o p) -> p o", p=P))
            nc.gpsimd.dma_start(w1_sb[:, 0:2], w1r[:, 0:2])
            nc.sync.dma_start(w1_sb[:, 2:4], w1r[:, 2:4])
            # chunks 2,3 feed the first matmuls of each v-group
            nc.scalar.dma_start(w8d23[:, 0, 0:1], w28br[:, 0, 0:1])
            nc.gpsimd.dma_start(w8d23[:, 0, 1:2], w28br[:, 0, 0:1])
            nc.scalar.dma_start(w8d23[:, 1, 0:1], w28br[:, 0, 1:2])
            nc.gpsimd.dma_start(w8d23[:, 1, 1:2], w28br[:, 0, 1:2])
            nc.sync.dma_start(w28d[:, 0, 0:1], w28r[:, 0, 0:1])
            nc.scalar.dma_start(w28d[:, 0, 1:2], w28r[:, 1, 0:1])
            nc.gpsimd.dma_start(w28d[:, 1, 0:1], w28r[:, 0, 1:2])
            nc.sync.dma_start(w28d[:, 1, 1:2], w28r[:, 1, 1:2])
            nc.sync.dma_start(wr8d[:], w28br[:, 1])

            # ---- projections (bf16 operands, fp32 psum) ----
            decbT = consts.tile([P, HT, U], F32)
            encbT = consts.tile([P, HT, T_SH], F32)
            # h-order (2,3,0,1): chunks 2/3 feed the first matmuls of u=0,
            # chunks 0/1 feed the Pool fp8 chains which have more slack.
            for i, h in enumerate((2, 3, 0, 1)):
                pj = (prA, prB)[i % 2]
                for d in range(DT):
                    nc.tensor.matmul(
                        pj[:, :U], w1_sb[:, DT + d, ts(h, P)], decT[:, :, d],
                        start=(d == 0), stop=(d == DT - 1),
                    )
                nc.vector.tensor_scalar_add(
                    decbT[:, h], pj[:, :U], b1_sb[:, h : h + 1]
                )
            for i, h in enumerate((2, 3, 0, 1)):
                pj = (prA, prB)[i % 2]
                for d in range(DT):
                    nc.tensor.matmul(
                        pj[:], w1_sb[:, d, ts(h, P)], encT[:, :, d],
                        start=(d == 0), stop=(d == DT - 1),
                    )
                nc.vector.tensor_copy(encbT[:, h], pj[:])

            # ---- main loop over u ----
            for u in range(U):
                hids = [None] * HT
                # tanh order: bf16 chunks (2,3) first - they feed the first
                # matmuls of each group - then the fp8 chunks (0,1) whose
                # Pool conversion chains run while the bf16 matmuls stream.
                for h in (2, 3, 0, 1):
                    ht = hidp.tile([P, T_SH], BF, tag="hid", name=f"t{h}")
                    nc.scalar.activation(
                        ht[:], encbT[:, h],
                        mybir.ActivationFunctionType.Tanh,
                        bias=decbT[:, h, u : u + 1], scale=1.0,
                    )
                    hids[h] = ht
                # chunks 2,3 first (they feed the first matmuls): planes
                # [h8_2, r8_2, h8_3, r8_3]; the cross matmul reads planes
                # (0,2) via a stride-2 slice.
                X = h8p.tile([P, 4, T_SH], F8, tag="hx", name="hx")
                nc.gpsimd.tensor_copy(X[:, 0], hids[2][:])
                nc.gpsimd.tensor_copy(X[:, 2], hids[3][:])
                nc.gpsimd.tensor_sub(X[:, 1], hids[2][:], X[:, 0])
                nc.gpsimd.tensor_sub(X[:, 3], hids[3][:], X[:, 2])
                h8r8 = []
                for c in range(NF8):
                    hr = h8p.tile([P, 2, T_SH], F8, tag="h8", name=f"h8r8{c}")
                    # plane0 = fp8(tanh), plane1 = fp8(tanh - plane0);
                    # both against fp8(W2*SW).  Weights are host-scaled by
                    # SW=8 (moves W2 fp8 values out of the subnormal range);
                    # the host descales the output exactly.
                    nc.gpsimd.tensor_copy(hr[:, 0], hids[c][:])
                    nc.gpsimd.tensor_sub(hr[:, 1], hids[c][:], hr[:, 0])
                    h8r8.append(hr)
                po = pso.tile([P, V], F32, tag="pso")
                so = ostage.tile([P, V], BF, tag="ostage")
                tail = u == U - 1
                widths = [512, 512] if not tail else [512, 256, 256]
                offs = [0, 512] if not tail else [0, 512, 768]
                nchunk = len(widths)
                # Interleave the two 512-wide v-groups (they accumulate in
                # different psum banks, so both can be open): all bf16
                # matmuls first, then the fp8 DoubleRows - gives the Pool
                # fp8-conversion chains an extra ~850ns of slack each u.
                # (The 256-wide tail chunks share banks: keep those serial.)
                if not tail:
                    for v in range(nchunk):
                        sl = ts(v, 512)
                        for i in range(2):
                            nc.tensor.matmul(
                                po[:, sl], X[:, 2 * i : 2 * i + 2],
                                w8d23[:, i, :, sl],
                                start=(i == 0), stop=False,
                                perf_mode=mybir.MatmulPerfMode.DoubleRow,
                            )
                    for v in range(nchunk):
                        sl = ts(v, 512)
                        nc.tensor.matmul(
                            po[:, sl], X[:, 0:4:2], wr8d[:, :, sl],
                            start=False, stop=False,
                            perf_mode=mybir.MatmulPerfMode.DoubleRow,
                        )
                        for c in range(NF8):
                            nc.tensor.matmul(
                                po[:, sl], h8r8[c][:], w28d[:, c, :, sl],
                                start=False, stop=(c == NF8 - 1),
                                perf_mode=mybir.MatmulPerfMode.DoubleRow,
                            )
                else:
                    for v in range(nchunk):
                        sl = slice(offs[v], offs[v] + widths[v])
                        for i in range(2):
                            nc.tensor.matmul(
                                po[:, sl], X[:, 2 * i : 2 * i + 2],
                                w8d23[:, i, :, sl],
                                start=(i == 0), stop=False,
                                perf_mode=mybir.MatmulPerfMode.DoubleRow,
                            )
                        nc.tensor.matmul(
                            po[:, sl], X[:, 0:4:2], wr8d[:, :, sl],
                            start=False, stop=False,
                            perf_mode=mybir.MatmulPerfMode.DoubleRow,
                        )
                        for c in range(NF8):
                            nc.tensor.matmul(
                                po[:, sl], h8r8[c][:], w28d[:, c, :, sl],
                                start=False, stop=(c == NF8 - 1),
                                perf_mode=mybir.MatmulPerfMode.DoubleRow,
                            )
                if not tail:
                    # output stays scaled by SW (the host descales by the
                    # exact power-of-two 1/SW after upconverting to f32).
                    # Near the tail, split the evacuation so DVE drains
                    # early and the last u's copies aren't queued out.
                    if u < U - 3:
                        nc.vector.tensor_copy(so[:], po[:])
                    else:
                        nc.vector.tensor_copy(so[:, :512], po[:, :512])
                        nc.vector.tensor_copy(so[:, 512:], po[:, 512:])
                    nc.sync.dma_start(out[u], so[:])
                else:
                    # tail: separate staging tiles (a shared one falsely
                    # serializes), copies on Pool chasing each chunk's stop
                    # (DVE is still draining u=62's evacuation), DMAs fan
                    # out across queues.
                    # (no DMAs on Pool here: a Pool-issued DMA holds the
                    # engine ~500ns for SWDGE descriptor generation, which
                    # would delay the chasing copies)
                    dma_eng = [nc.scalar, nc.sync, nc.scalar]
                    for v in range(nchunk):
                        sl = slice(offs[v], offs[v] + widths[v])
                        sov = ostage.tile(
                            [P, widths[v]], BF, tag=f"sot{v}", name=f"sov{v}"
                        )
                        nc.vector.tensor_copy(sov[:], po[:, sl])
                        dma_eng[v].dma_start(out[u, :, sl], sov[:])
    return nc


_NC_CACHE = None


def _get_nc():
    global _NC_CACHE
    if _NC_CACHE is None:
        _NC_CACHE = build_nc()
    return _NC_CACHE


def _q8(x):
    return x.astype(_f8).astype(np.float32)


def _qb(x):
    return x.astype(_bf16).astype(np.float32)


def _lane_order(enc, dec, W1, b1, W2):
    """Rank H lanes by E[tanh^2] * ||fp8 err of W2 row||^2 (ascending =
    best fp8 candidates).  Sampled over every 4th t for speed."""
    ep = _qb(enc.reshape(-1, D)) @ _qb(W1[:D])
    dp = _qb(dec.reshape(-1, D)) @ _qb(W1[D:])
    ep = ep.reshape(B, T, H)[:, ::4]
    dp = dp.reshape(B, U, H)
    hs = np.tanh(ep[:, :, None, :] + dp[:, None, :, :] + b1)
    Eh2 = (hs * hs).mean(axis=(0, 1, 2))
    w2err = _q8(W2 * SW) / SW - W2
    score = Eh2 * (w2err * w2err).sum(axis=1)
    return np.argsort(score)


def prepare_weights(W1, b1, W2, order):
    """Permute the H axis and build the device weight arrays."""
    sel = np.sort(order[: NF8 * P])
    rest = np.sort(order[NF8 * P :])
    perm = np.concatenate([sel, rest])
    W1p = np.ascontiguousarray(W1[:, perm]).astype(_bf16)
    b1p = np.ascontiguousarray(b1[perm])
    w28p = (W2[sel] * SW).astype(_f8)
    w28 = np.stack([w28p, w28p])
    w8_23 = (W2[rest] * SW).astype(_f8)
    wr8_23 = (W2[rest] * SW - w8_23.astype(np.float32)).astype(_f8)
    w28b = np.stack([w8_23, wr8_23])
    return W1p, b1p, w28, w28b


def kernel(encoder_outputs, decoder_outputs, W1, b1, W2):
    encoder_outputs = np.asarray(encoder_outputs, dtype=np.float32)
    decoder_outputs = np.asarray(decoder_outputs, dtype=np.float32)
    W1 = np.ascontiguousarray(np.asarray(W1, dtype=np.float32))
    b1 = np.ascontiguousarray(np.asarray(b1, dtype=np.float32))
    W2 = np.ascontiguousarray(np.asarray(W2, dtype=np.float32))

    order = _lane_order(encoder_outputs, decoder_outputs, W1, b1, W2)
    W1p, b1p, w28, w28b = prepare_weights(W1, b1, W2, order)

    nc = _get_nc()
    in_maps = []
    for c in range(N_CORES):
        b, th = divmod(c, T // T_SH)
        in_maps.append(
            {
                "enc": np.ascontiguousarray(
                    encoder_outputs[b, th * T_SH : (th + 1) * T_SH]
                ).astype(_bf16),
                "dec": np.ascontiguousarray(decoder_outputs[b]).astype(_bf16),
                "w1": W1p,
                "b1": b1p,
                "w28": w28,
                "w28b": w28b,
            }
        )
    res = run_bass_kernel_spmd(nc, in_maps, core_ids=list(range(N_CORES)))
    out = np.empty((B, T, U, V), np.float32)
    for c in range(N_CORES):
        b, th = divmod(c, T // T_SH)
        # device layout is [U, T_SH, V] bf16; swap to [T_SH, U, V] f32
        out[b, th * T_SH : (th + 1) * T_SH] = (
            res.results[c]["out"].astype(np.float32).transpose(1, 0, 2)
            * np.float32(1.0 / SW)
        )
    return out
